# revision 41
# baseline (speedup 1.0000x reference)
"""GPS layer (GCN + per-graph MHA + FFN, BatchNorm eval) on 8 trn2 cores.

Sharding: 16 graphs data-parallel, 2 graphs per core (block-diagonal
adjacency => no cross-core edges). Each core runs an identical Bass/Tile
program on its slice.

Design notes (cost model: matmul cost = out-free-size x pe_cycle x
cycles/row, independent of K and M; fp8 DoubleRow = 0.5 cycles/row with
two K-blocks per call):
- Activations feature-major [d, n] everywhere except inside attention.
- fp8e4m3 + DoubleRow matmuls for GCN (reassociated as Wg @ (x^T A^T)),
  QKV, attn@V, FFN1, FFN2 (host pre-scales weights into fp8 range;
  descale constants fold into downstream ACT/DVE affine ops).
- Attention: scores^T = K^T q per (g,h,kb) in bf16 (K=32, no DR
  pairing possible); exp on ACT reads 2 PSUM banks per instr, output
  *16 via bias=ln(16), written directly as fp8; a few exp pairs per
  graph run on DVE via 16*e^s ~ (4+2s)^2 (scores are small) to keep
  ACT, the critical engine, fed.
- attn@V runs node-major: out[q, dh] with N=34 free (32 dh + Z col +
  pad), so the softmax denominator Z lands per-partition; normalize is
  one DVE reciprocal [128,8] + one broadcast multiply per (g,qb).
- ctx transposed back to feature-major with PE transpose ops for the
  out-proj (4 blocks per bf16 PSUM bank).
- All BatchNorm/bias algebra precomputed on host into per-partition
  scale/shift vectors; residual+BN fusions are single DVE/Pool
  scalar_tensor_tensor ops: x_out = (psum * s) + carrier, with the
  carriers (x*s1+t1 etc.) computed on host.
- ACT table switches limited to gelu -> exp -> gelu; the exp table
  load is hidden in a pre-scores bubble, and FFN1 gelus read a bias
  vector data-dependent on the last exp tiles so the tile scheduler
  cannot interleave them into the exp stream (each interleave would
  cost two table reloads).
- Everything after scores is emitted per graph so g0's attn@V /
  out-proj / FFN1 run (in-order engines) while ACT still exps g1;
  PSUM tags are assigned per graph to avoid WAR rotation stalls.
- Weight DMAs for late phases issue from the idle GPSIMD queue.
"""

import numpy as np
import ml_dtypes

BF16 = ml_dtypes.bfloat16
FP8 = ml_dtypes.float8_e4m3

B, N, D, H = 16, 512, 256, 8
EP = 16384
NCORES = 8
GPC = B // NCORES            # graphs per core = 2
NODES = N * GPC              # nodes per core = 1024
DH = D // H                  # 32
BN_EPS = 1e-5
C_ATT = float(1.0 / np.sqrt(DH))

# fp8 scale factors (host-side); descales folded into device affines.
SX = 16.0    # x fp8
SW = 16.0    # w_gcn fp8
SH = 16.0    # hl fp8
SA = 64.0    # adjacency values fp8
SX1 = 16.0   # x1 fp8
SIW = 16.0   # in_proj_w fp8
SE = 16.0    # exp(scores) fp8
SV = 16.0    # v fp8
ZC = 1.0 / 32.0  # Z-column value in v_aug
SW1 = 16.0   # w1 fp8
SX2 = 16.0   # x2 fp8
SW2 = 16.0   # w2 fp8
SCTX = SV / ZC  # ctx_norm carries 512*ctx

# cv columns (per-feature constant vectors, [128, col, db])
CV_S1SX1 = 0   # s1*SX1
CV_S1S2 = 1    # s1*s2
CV_S2O = 2     # s2/SCTX
CV_S3W = 3     # s3/SW2
NCV = 4

_prog_cache = {}
DEBUG_TAPS = False
DVE_EXP_SET = {(2, 1), (4, 1), (6, 1)}
DVE_EXP_SET3 = {(g, h, j) for g in range(2) for (h, j) in DVE_EXP_SET}


def _split_waits(nc, mybir, max_waits=1):
    """walrus CoreV3 rejects >1 sync wait per instruction; move excess
    waits onto preceding NOPs."""
    for bb in nc.main_func.blocks:
        new_instrs = []
        for ins in bb.instructions:
            si = ins.sync_info
            waits = list(si.on_wait) if si is not None and si.on_wait else []
            if len(waits) > max_waits:
                keep = waits[-max_waits:]
                for i, w in enumerate(waits[:-max_waits]):
                    new_instrs.append(
                        mybir.InstNoOp(
                            name=f"{ins.name}-ws{i}",
                            sync_info=mybir.SyncInfo(on_wait=[w], on_update=[]),
                            bass_nofuse=True,
                            engine=ins.engine,
                        )
                    )
                ins.sync_info = mybir.SyncInfo(
                    on_wait=keep, on_update=list(si.on_update or [])
                )
            new_instrs.append(ins)
        bb.instructions[:] = new_instrs


def _build_program():
    import concourse.bass as bass
    import concourse.tile as tile
    import concourse.mybir as mybir

    f32 = mybir.dt.float32
    bf = mybir.dt.bfloat16
    f8 = mybir.dt.float8e4
    AF = mybir.ActivationFunctionType
    OP = mybir.AluOpType
    DR = mybir.MatmulPerfMode.DoubleRow

    nc = bass.Bass()
    dp = nc.declare_dram_parameter
    # all params are pre-laid-out on host to the exact SBUF tile shape
    x8n = dp("x8n", [128, 8, D], f8, isOutput=False)
    wg8 = dp("wg8", [128, 2, D], f8, isOutput=False)
    a8 = dp("a8", [128, 8, N], f8, isOutput=False)
    cv = dp("cv", [128, NCV, 2], f32, isOutput=False)
    xs1s = dp("xs1s", [128, 2, NODES], bf, isOutput=False)   # (x*s1+t1)*SX1
    xs12 = dp("xs12", [128, 2, NODES], bf, isOutput=False)   # (x*s1+t1)*s2+c2
    ipw8 = dp("ipw8", [128, 2, 3 * D], f8, isOutput=False)
    ipbd = dp("ipbd", [128, 4], f32, isOutput=False)
    opwt = dp("opwt", [128, 2, D], bf, isOutput=False)
    w18 = dp("w18", [128, 2, 4 * D], f8, isOutput=False)
    b1d = dp("b1d", [128, 8], f32, isOutput=False)
    w28 = dp("w28", [128, 8, D], f8, isOutput=False)
    xc3 = dp("xc3", [128, 2], f32, isOutput=False)           # s3 per db col
    tc3 = dp("tc3", [128, 2], f32, isOutput=False)           # t3+b2*s3 col
    identb = dp("identb", [128, 128], bf, isOutput=False)
    outp = dp("out", [128, 2, NODES], f32, isOutput=True)
    if DEBUG_TAPS:
        d_m18 = dp("d_m18", [128, 2, GPC, N], f8, isOutput=True)
        d_gl = dp("d_gl", [128, 2, NODES], bf, isOutput=True)
        d_x18 = dp("d_x18", [128, 2, NODES], f8, isOutput=True)
        d_xs2 = dp("d_xs2", [128, 2, NODES], bf, isOutput=True)
        d_qk = dp("d_qk", [128, 4, GPC, N], bf, isOutput=True)
        d_va = dp("d_va", [128, GPC, 4, H, 34], f8, isOutput=True)
        d_es = dp("d_es", [128, GPC, H, 4, N], f8, isOutput=True)
        d_cn = dp("d_cn", [128, GPC, 4, D], bf, isOutput=True)
        d_ctxT = dp("d_ctxT", [128, 2, GPC, N], bf, isOutput=True)
        d_x2 = dp("d_x2", [128, 2, NODES], bf, isOutput=True)
        d_h18 = dp("d_h18", [128, 8, NODES], f8, isOutput=True)

    LOG_SE = float(np.log(SE))

    with tile.TileContext(nc) as tc:
        with (
            tc.tile_pool(name="const", bufs=1) as cp,
            tc.tile_pool(name="act", bufs=1) as ap_,
            tc.tile_pool(name="work", bufs=2) as wp,
            tc.tile_pool(name="pbig", bufs=2, space="PSUM") as pb,
            tc.tile_pool(name="psmall", bufs=2, space="PSUM") as ps_,
        ):
            # ---------- constant loads ----------
            t_x8n = cp.tile([128, 8, D], f8, tag="x8n")
            nc.sync.dma_start(t_x8n[:, 0:4, :], x8n[:, 0:4, :])
            t_a8 = cp.tile([128, 8, N], f8, tag="a8")
            nc.sync.dma_start(t_a8[:, 0:4, :], a8[:, 0:4, :])
            t_wg8 = cp.tile([128, 2, D], f8, tag="wg8")
            nc.sync.dma_start(t_wg8[:], wg8[:])
            nc.sync.dma_start(t_x8n[:, 4:8, :], x8n[:, 4:8, :])
            nc.sync.dma_start(t_a8[:, 4:8, :], a8[:, 4:8, :])
            t_cv = cp.tile([128, NCV, 2], f32, tag="cv")
            nc.sync.dma_start(t_cv[:], cv[:])
            t_xs1s = cp.tile([128, 2, NODES], bf, tag="xs1s")
            nc.sync.dma_start(t_xs1s[:], xs1s[:])
            t_xs12 = cp.tile([128, 2, NODES], bf, tag="xs12")
            nc.sync.dma_start(t_xs12[:], xs12[:])
            t_ipw8 = cp.tile([128, 2, 3 * D], f8, tag="ipw8")
            nc.sync.dma_start(t_ipw8[:], ipw8[:])
            t_ipbd = cp.tile([128, 4], f32, tag="ipbd")
            nc.sync.dma_start(t_ipbd[:], ipbd[:])
            t_opwt = cp.tile([128, 2, D], bf, tag="opwt")
            nc.gpsimd.dma_start(t_opwt[:], opwt[:])
            t_w18 = cp.tile([128, 2, 4 * D], f8, tag="w18")
            nc.gpsimd.dma_start(t_w18[:], w18[:])
            t_b1d = cp.tile([128, 8], f32, tag="b1d")
            nc.gpsimd.dma_start(t_b1d[:], b1d[:])
            t_w28 = cp.tile([128, 8, D], f8, tag="w28")
            nc.gpsimd.dma_start(t_w28[:], w28[:])
            t_xc3 = cp.tile([128, 2], f32, tag="xc3")
            nc.gpsimd.dma_start(t_xc3[:], xc3[:])
            t_tc3 = cp.tile([128, 2], f32, tag="tc3")
            nc.gpsimd.dma_start(t_tc3[:], tc3[:])
            t_id = cp.tile([128, 128], bf, tag="identb")
            nc.gpsimd.dma_start(t_id[:], identb[:])
            t_lse = cp.tile([128, 1], f32, tag="lse")
            nc.vector.memset(t_lse[:], LOG_SE)
            # make the first ACT op a Gelu so the initial (free) table load
            # fetches the gelu table; the GCN gelus then need no load
            t_scr0 = wp.tile([128, 1], f32, tag="scr0")
            nc.scalar.activation(t_scr0[:], t_lse[:], AF.Gelu)

            # ---------- persistent activations ----------
            t_m18 = ap_.tile([128, 2, GPC, N], f8, tag="m18")
            t_gl = ap_.tile([128, 2, NODES], bf, tag="gl")
            t_x18 = ap_.tile([128, 2, NODES], f8, tag="x18")
            t_xs2 = ap_.tile([128, 2, NODES], bf, tag="xs2")
            t_qk = ap_.tile([128, 4, GPC, N], bf, tag="qk")
            t_va = ap_.tile([128, GPC, 4, H, 34], f8, tag="va")
            t_es = ap_.tile([128, GPC, H, 4, N], f8, tag="es")
            t_cn = ap_.tile([128, GPC, 4, D], bf, tag="cn")
            t_ctxT = ap_.tile([128, 2, GPC, N], bf, tag="ctxT")
            t_x2 = ap_.tile([128, 2, NODES], bf, tag="x2")
            t_x28 = ap_.tile([128, 2, NODES], f8, tag="x28")
            t_xs3 = ap_.tile([128, 2, NODES], bf, tag="xs3")
            t_h18 = ap_.tile([128, 8, NODES], f8, tag="h18")
            t_out = ap_.tile([128, 2, NODES], f32, tag="outT")

            # v_aug constant columns: col 32 = ZC (Z accumulator), col 33 = 0
            nc.vector.memset(t_va[:, :, :, :, 33:34], 0.0)
            nc.vector.memset(t_va[:, :, :, :, 32:33], ZC)

            # ---------- GCN: agg^T = Wg (x^T A^T), per graph ----------
            for g in range(GPC):
                ns = slice(g * N, (g + 1) * N)
                for db in range(2):
                    ps = ps_.tile([128, 512], f32, space="PSUM", tag="ps1")
                    for i in range(2):
                        nc.tensor.matmul(
                            ps[:],
                            t_x8n[:, 4 * g + 2 * i:4 * g + 2 * i + 2,
                                  db * 128:(db + 1) * 128],
                            t_a8[:, 4 * g + 2 * i:4 * g + 2 * i + 2, :],
                            start=(i == 0), stop=(i == 1), perf_mode=DR,
                        )
                    if db == 0:
                        nc.scalar.activation(
                            t_m18[:, db, g, :], ps[:], AF.Copy,
                            scale=SH / (SX * SA),
                        )
                    else:
                        nc.vector.tensor_scalar_mul(
                            t_m18[:, db, g, :], ps[:], SH / (SX * SA)
                        )
                for db in range(2):
                    ps = ps_.tile([128, 512], f32, space="PSUM", tag="ps1")
                    nc.tensor.matmul(
                        ps[:],
                        t_wg8[:, :, db * 128:(db + 1) * 128],
                        t_m18[:, :, g, :],
                        start=True, stop=True, perf_mode=DR,
                    )
                    nc.scalar.activation(
                        t_gl[:, db, ns], ps[:], AF.Gelu,
                        scale=1.0 / (SH * SW),
                    )
                    # x1*SX1 in fp8: (gl * s1*SX1) + (x*s1+t1)*SX1
                    nc.vector.scalar_tensor_tensor(
                        t_x18[:, db, ns], t_gl[:, db, ns],
                        t_cv[:, CV_S1SX1, db:db + 1], t_xs1s[:, db, ns],
                        OP.mult, OP.add,
                    )

            # ---------- QKV projections (fp8 DoubleRow) ----------
            for g in range(GPC):
                ns = slice(g * N, (g + 1) * N)
                for eb in (0, 2, 1, 3):   # h0-3 need eb0(q)+eb2(k) first
                    ps = ps_.tile([128, 512], f32, space="PSUM", tag="ps1")
                    nc.tensor.matmul(
                        ps[:],
                        t_ipw8[:, :, eb * 128:(eb + 1) * 128],
                        t_x18[:, :, ns],
                        start=True, stop=True, perf_mode=DR,
                    )
                    # q block already carries 1/sqrt(dh) via host ipw scaling
                    if g == 0 and eb in (0, 2):
                        # ACT is idle in this window; parallelize with DVE
                        nc.scalar.activation(
                            t_qk[:, eb, g, :], ps[:], AF.Identity,
                            scale=1.0 / (SIW * SX1),
                            bias=t_ipbd[:, eb:eb + 1],
                        )
                    else:
                        nc.vector.tensor_scalar(
                            t_qk[:, eb, g, :], ps[:],
                            1.0 / (SIW * SX1), t_ipbd[:, eb:eb + 1],
                            OP.mult, OP.add,
                        )
                for nb in range(4):
                    nlo = g * N + nb * 128
                    ps = ps_.tile([128, 512], f32, space="PSUM", tag="ps1")
                    nc.tensor.matmul(
                        ps[:, 0:D],
                        t_x18[:, :, nlo:nlo + 128],
                        t_ipw8[:, :, 2 * D:3 * D],
                        start=True, stop=True, perf_mode=DR,
                    )
                    nc.vector.tensor_scalar_mul(
                        t_va[:, g, nb, :, 0:DH],
                        ps[:, 0:D].rearrange("p (h d) -> p h d", h=H),
                        SV / (SIW * SX1),
                    )
                if g == 0:
                    # preload the exp ACT table in the pre-scores bubble
                    t_scr = wp.tile([128, 1], f32, tag="scr")
                    nc.scalar.activation(
                        t_scr[:], t_gl[:, 1, 1023:1024], AF.Exp)
                    # residual-2 carrier (needed only at out-proj time)
                    for db in range(2):
                        nc.vector.scalar_tensor_tensor(
                            t_xs2[:, db, :], t_gl[:, db, :],
                            t_cv[:, CV_S1S2, db:db + 1], t_xs12[:, db, :],
                            OP.mult, OP.add,
                        )

            # ---------- scores + exp (per graph, head) ----------
            for g in range(GPC):
                for h in range(H):
                    hb, po = h // 4, 32 * (h % 4)
                    for j in range(2):
                        ps = pb.tile([128, 2, N], f32, space="PSUM", tag="ps2")
                        for i in range(2):
                            kb = 2 * j + i
                            nc.tensor.matmul(
                                ps[:, i, :],
                                t_qk[po:po + 32, 2 + hb, g,
                                     kb * 128:(kb + 1) * 128],
                                t_qk[po:po + 32, hb, g, :],
                                start=True, stop=True,
                                tile_position=(po, 0),
                                skip_group_check=True,
                            )
                        if (g, h, j) in DVE_EXP_SET3:
                            # DVE exp approx: 16*e^s ~ (4 + 2s)^2
                            t_eu = wp.tile([128, 2, N], bf, tag="eu")
                            nc.vector.tensor_scalar(
                                t_eu[:], ps[:], 2.0, 4.0, OP.mult, OP.add,
                            )
                            nc.vector.tensor_tensor(
                                t_es[:, g, h, 2 * j:2 * j + 2, :],
                                t_eu[:], t_eu[:], OP.mult,
                            )
                        else:
                            nc.scalar.activation(
                                t_es[:, g, h, 2 * j:2 * j + 2, :],
                                ps[:].rearrange("p a n -> p (a n)"),
                                AF.Exp, bias=t_lse[:],
                            )

            # bias token: numerically equals b1d, but depends on the last
            # exp tiles so the scheduler cannot run FFN1 gelus mid-exp
            # (each interleave costs two activation-table reloads)
            t_b1tok = ap_.tile([128, 8], f32, tag="b1tok")
            nc.vector.scalar_tensor_tensor(
                t_b1tok[:], t_es[:, GPC - 1, :, 3, 0:1], 0.0, t_b1d[:],
                OP.mult, OP.add,
            )

            # ---------- per-graph post-attention pipeline ----------
            # Engines run in program order, so everything for g0 (attn@V,
            # transpose, out-proj, FFN1 matmuls) is issued before anything
            # of g1: the g0 chain runs while ACT is still exp-ing g1.
            for g in range(GPC):
                ns = slice(g * N, (g + 1) * N)
                # attn@V node-major + normalize
                for qb in range(4):
                    pc = ps_.tile([128, 512], f32, space="PSUM", tag="pc")
                    pcv = pc[:, 0:H * 34].rearrange("p (h d) -> p h d", h=H)
                    for h in range(H):
                        for i in range(2):
                            nc.tensor.matmul(
                                pcv[:, h, :],
                                t_es[:, g, h, 2 * i:2 * i + 2,
                                     qb * 128:(qb + 1) * 128],
                                t_va[:, g, 2 * i:2 * i + 2, h, :],
                                start=(h == 0 and i == 0),
                                stop=(h == H - 1 and i == 1),
                                perf_mode=DR,
                                skip_group_check=True,
                            )
                    t_rz = wp.tile([128, H, 1], f32, tag="rz")
                    nc.vector.reciprocal(t_rz[:], pcv[:, :, 32:33])
                    nc.vector.tensor_tensor(
                        t_cn[:, g, qb, :].rearrange("p (h d) -> p h d", h=H),
                        pcv[:, :, 0:DH],
                        t_rz[:].broadcast_to((128, H, DH)),
                        OP.mult,
                    )
                # transpose ctx to feature-major
                for db in (0, 1):
                    pt = ps_.tile([128, 1024], bf, space="PSUM", tag="pc",
                                  name=f"pt{g}{db}")
                    for qb in range(4):
                        nc.tensor.matmul(
                            pt[:, qb * 128:(qb + 1) * 128],
                            t_cn[:, g, qb, db * 128:(db + 1) * 128],
                            t_id[:],
                            is_transpose=True,
                            start=(qb == 0), stop=(qb == 3),
                            skip_group_check=True,
                        )
                    nc.vector.tensor_copy(
                        t_ctxT[:, db, g, :], pt[:, 0:512]
                    )
                # out-proj + residual + BN2
                for eb in range(2):
                    ps = ps_.tile([128, 512], f32, space="PSUM",
                                  tag="ps1" if g == 0 else "pc")
                    for kd in range(2):
                        nc.tensor.matmul(
                            ps[:],
                            t_opwt[:, kd, eb * 128:(eb + 1) * 128],
                            t_ctxT[:, kd, g, :],
                            start=(kd == 0), stop=(kd == 1),
                        )
                    # x2 = psum * (s2/SCTX) + (x1*s2 + t2 + opb_eff*s2)
                    nc.vector.scalar_tensor_tensor(
                        t_x2[:, eb, ns], ps[:],
                        t_cv[:, CV_S2O, eb:eb + 1], t_xs2[:, eb, ns],
                        OP.mult, OP.add,
                    )
                # fp8 copy for FFN1 rhs: DVE for g1 (critical chain to
                # the last gelus); residual-3 carrier stays on Pool
                eng28 = nc.gpsimd if g == 0 else nc.vector
                for db in range(2):
                    eng28.tensor_scalar_mul(
                        t_x28[:, db, ns], t_x2[:, db, ns], SX2
                    )
                    nc.gpsimd.tensor_scalar(
                        t_xs3[:, db, ns], t_x2[:, db, ns],
                        t_xc3[:, db:db + 1], t_tc3[:, db:db + 1],
                        OP.mult, OP.add,
                    )
                # FFN1 matmuls + gelu (gelus run on ACT after the exp
                # stream drains; matmuls for g0 fire much earlier)
                for mb in range(8):
                    ps = ps_.tile([128, 512], f32, space="PSUM",
                                  tag="ps1" if g == 0 else "pc")
                    nc.tensor.matmul(
                        ps[:],
                        t_w18[:, :, mb * 128:(mb + 1) * 128],
                        t_x28[:, :, ns],
                        start=True, stop=True, perf_mode=DR,
                    )
                    nc.scalar.activation(
                        t_h18[:, mb, ns], ps[:], AF.Gelu,
                        scale=1.0 / (SW1 * SX2), bias=t_b1tok[:, mb:mb + 1],
                    )

            # ---------- FFN2 (fp8 DR), dep-driven tail ----------
            ps2f = [pb.tile([128, 2, N], f32, space="PSUM", tag="ps2",
                            name=f"ps2f{_g}")
                    for _g in range(GPC)]
            for g in range(GPC):
                ns = slice(g * N, (g + 1) * N)
                for jj in range(4):
                    for db in range(2):
                        nc.tensor.matmul(
                            ps2f[g][:, db, :],
                            t_w28[:, 2 * jj:2 * jj + 2,
                                  db * 128:(db + 1) * 128],
                            t_h18[:, 2 * jj:2 * jj + 2, ns],
                            start=(jj == 0), stop=(jj == 3),
                            perf_mode=DR,
                        )
                for db in range(2):
                    nc.vector.scalar_tensor_tensor(
                        t_out[:, db, ns], ps2f[g][:, db, :],
                        t_cv[:, CV_S3W, db:db + 1], t_xs3[:, db, ns],
                        OP.mult, OP.add,
                    )
                    nc.sync.dma_start(outp[:, db, ns], t_out[:, db, ns])
            if DEBUG_TAPS:
                for dd, tl in [(d_m18, t_m18), (d_gl, t_gl),
                               (d_x18, t_x18), (d_xs2, t_xs2),
                               (d_qk, t_qk), (d_va, t_va),
                               (d_es, t_es), (d_cn, t_cn),
                               (d_ctxT, t_ctxT), (d_x2, t_x2),
                               (d_h18, t_h18)]:
                    nc.sync.dma_start(dd[:], tl[:])

    _split_waits(nc, mybir, 1)
    return nc


def _host_prep(inputs):
    """Build per-core input maps with everything pre-laid-out."""
    x = np.asarray(inputs["x"], np.float32)
    er = np.asarray(inputs["edge_rows"]).astype(np.int64)
    ec = np.asarray(inputs["edge_cols"]).astype(np.int64)
    ev = np.asarray(inputs["edge_vals"], np.float32)

    ipw = np.asarray(inputs["in_proj_w"], np.float32)
    ipb = np.asarray(inputs["in_proj_b"], np.float32)
    opw = np.asarray(inputs["out_proj_w"], np.float32)
    opb = np.asarray(inputs["out_proj_b"], np.float32)
    w1 = np.asarray(inputs["w1"], np.float32)
    b1 = np.asarray(inputs["b1"], np.float32)
    w2 = np.asarray(inputs["w2"], np.float32)
    b2 = np.asarray(inputs["b2"], np.float32)

    s = {}
    t = {}
    for k in (1, 2, 3):
        g_ = np.asarray(inputs[f"bn{k}_g"], np.float32)
        b_ = np.asarray(inputs[f"bn{k}_b"], np.float32)
        m_ = np.asarray(inputs[f"bn{k}_m"], np.float32)
        v_ = np.asarray(inputs[f"bn{k}_v"], np.float32)
        s[k] = g_ / np.sqrt(v_ + BN_EPS)
        t[k] = b_ - m_ * s[k]

    opb_eff = opb + opw @ ipb[2 * D:3 * D]
    c2 = t[2] + opb_eff * s[2]
    c3 = t[3] + b2 * s[3]

    def bycol(vec, ncol):
        # [ncol*128] -> [128, ncol]
        return np.ascontiguousarray(vec.reshape(ncol, 128).T)

    def kmaj(w, scale, dt):
        # w [out, k] -> [128, k//128, out] with k = 128*i + p
        k = w.shape[1]
        return np.ascontiguousarray(
            (w.T * scale).reshape(k // 128, 128, w.shape[0]).transpose(1, 0, 2)
        ).astype(dt)

    cvh = np.stack([s[1] * SX1, s[1] * s[2], s[2] / SCTX, s[3] / SW2])
    cv = np.ascontiguousarray(
        cvh.reshape(NCV, 2, 128).transpose(2, 0, 1)).astype(np.float32)

    ipw_sc = ipw.copy()
    ipw_sc[0:D] *= C_ATT          # fold 1/sqrt(dh) into q projection
    ipb_eff = ipb[0:2 * D].copy()
    ipb_eff[0:D] *= C_ATT

    shared = {
        "wg8": kmaj(np.asarray(inputs["w_gcn"], np.float32), SW, FP8),
        "cv": cv,
        "ipw8": kmaj(ipw_sc, SIW, FP8),
        "ipbd": bycol(ipb_eff, 4).astype(np.float32),
        "opwt": kmaj(opw, 1.0, BF16),
        "w18": kmaj(w1, SW1, FP8),
        "b1d": bycol(b1, 8).astype(np.float32),
        "w28": kmaj(w2, SW2, FP8),
        "xc3": bycol(s[3], 2).astype(np.float32),
        "tc3": bycol(c3, 2).astype(np.float32),
        "identb": np.eye(128, dtype=np.float32).astype(BF16),
    }

    def featmaj(arr_dn, dt, scale=1.0):
        # [nodes, D] -> [128, 2, nodes] with d = 128*a + p
        a = (arr_dn.T * scale).reshape(2, 128, arr_dn.shape[0])
        return np.ascontiguousarray(a.transpose(1, 0, 2)).astype(dt)

    in_maps = []
    for c in range(NCORES):
        base = c * NODES
        elo, ehi = GPC * c * EP, GPC * (c + 1) * EP
        r = (er[elo:ehi] - base).astype(np.int64)
        cc = (ec[elo:ehi] - base).astype(np.int64)
        v = ev[elo:ehi]
        at = np.zeros((NODES, N), np.float32)
        np.add.at(at, (cc, r % N), v)
        a8 = np.ascontiguousarray(
            (at * SA).reshape(8, 128, N).transpose(1, 0, 2)).astype(FP8)
        xc = x[base:base + NODES]                       # [1024, 256]
        xs1s_h = (xc * s[1] + t[1]) * SX1
        xs12_h = (xc * s[1] + t[1]) * s[2] + c2
        in_maps.append(
            {
                "x8n": np.ascontiguousarray(
                    (xc * SX).reshape(8, 128, D).transpose(1, 0, 2)
                ).astype(FP8),
                "a8": a8,
                "xs1s": featmaj(xs1s_h, BF16),
                "xs12": featmaj(xs12_h, BF16),
                **shared,
            }
        )
    return in_maps


def kernel(**inputs):
    from concourse.bass_utils import run_bass_kernel_spmd

    in_maps = _host_prep(inputs)

    if "prog" not in _prog_cache:
        _prog_cache["prog"] = _build_program()
    nc = _prog_cache["prog"]
    _prog_cache["last_in_maps"] = in_maps

    res = run_bass_kernel_spmd(nc, in_maps, list(range(NCORES)))
    out = np.empty((B * N, D), np.float32)
    for c in range(NCORES):
        o = res.results[c]["out"]                        # [128, 2, 1024]
        out[c * NODES:(c + 1) * NODES] = (
            o.transpose(1, 0, 2).reshape(D, NODES).T
        )
    return out
